# revision 17
# baseline (speedup 1.0000x reference)
"""Trainium2 Bass kernel for nn_ContinuousEmbedding (masked matmul + bias).

Computes out = x @ (weights * mask) + bias, reshaped to [B, in_size, out_size],
where mask zeroes each input feature's own [out_size]-wide diagonal block.

Strategy: tensor-parallel across the 8 NeuronCores by splitting the
in_size*out_size (=16384) output columns into 8 shards of 2048 columns.
Each core gets the full x (as x^T for the matmul's stationary operand),
its weight-column shard (mask is constant — folded into the weights on the
host), and its bias shard. Per core: out_shard = x @ W_shard + bias_shard
via 128x128 fp32 matmuls accumulating K=256 as 2 PSUM passes; bias-add is
fused into the PSUM->SBUF eviction on the vector engine.
"""

import numpy as np

B = 4096
IN_SIZE = 256
OUT_SIZE = 64
IO = IN_SIZE * OUT_SIZE          # 16384
N_CORES = 8
N_SHARD = IO // N_CORES          # 2048 output columns per core
P = 128                          # SBUF partitions
KO = IN_SIZE // P                # 2 contraction sub-tiles
N_TILE = 512                     # matmul moving free dim (fp32 max)
M_TILES = B // P                 # 32 output row tiles
N_TILES = N_SHARD // N_TILE      # 4 column tiles per core

MATMUL_MODE = "fp32r"            # "fp32" | "fp32r" | "fp32x3"

_CACHE: dict = {}


def _build_program(mode=None):
    mode = mode or MATMUL_MODE
    import concourse.mybir as mybir
    import concourse.tile as tile
    import concourse.bass as bass
    from concourse import bacc

    nsplit = 2 if mode == "fp32x3" else 1
    nc = bacc.Bacc(
        "TRN2", target_bir_lowering=False, debug=False, num_devices=N_CORES
    )
    xt = nc.dram_tensor(
        "xt", [nsplit, IN_SIZE, B], mybir.dt.float32, kind="ExternalInput"
    ).ap()
    w = nc.dram_tensor(
        "w", [nsplit, IN_SIZE, N_SHARD], mybir.dt.float32, kind="ExternalInput"
    ).ap()
    bias = nc.dram_tensor(
        "bias", [N_SHARD], mybir.dt.float32, kind="ExternalInput"
    ).ap()
    out = nc.dram_tensor(
        "out", [B, N_SHARD], mybir.dt.float32, kind="ExternalOutput"
    ).ap()

    with tile.TileContext(nc) as tc:
        with tc.tile_pool(name="const", bufs=1) as const, \
             tc.tile_pool(name="psum", bufs=2, space="PSUM") as psum_pool, \
             tc.tile_pool(name="outp", bufs=3) as outp:
            mm_dt = (mybir.dt.float32 if mode == "fp32"
                     else mybir.dt.float32r)
            w_sb = const.tile([P, nsplit, KO, N_SHARD], mm_dt)
            xt_sb = const.tile([P, nsplit, KO, B], mm_dt)
            bias_sb = const.tile([P, N_SHARD], mybir.dt.float32)

            ld_eng = nc.sync if mode == "fp32" else nc.gpsimd
            ld_eng.dma_start(
                out=w_sb[:], in_=w.rearrange("s (ko p) n -> p s ko n", p=P)
            )
            # bias [N_SHARD] broadcast across all 128 partitions (stride-0).
            bias_bcast = bass.AP(
                tensor=bias.tensor,
                offset=bias.offset,
                ap=[[0, P]] + list(bias.ap),
            )
            nc.sync.dma_start(out=bias_sb[:], in_=bias_bcast)
            # x^T load split into chunks so early m-tiles can start sooner.
            xt_src = xt.rearrange("s (ko p) m -> p s ko m", p=P)
            n_chunks = 4
            chunk = B // n_chunks
            for c in range(n_chunks):
                sl = slice(c * chunk, (c + 1) * chunk)
                ld_eng.dma_start(out=xt_sb[:, :, :, sl], in_=xt_src[:, :, :, sl])

            for m in range(M_TILES):
                out_sb = outp.tile([P, N_SHARD], mybir.dt.float32)
                ps = [psum_pool.tile([P, N_TILE], mybir.dt.float32,
                                     name=f"ps{n}", tag=f"ps{n}")
                      for n in range(N_TILES)]
                # (x_split, w_split) matmul terms: plain modes use (0,0);
                # fp32x3 adds the hi/lo cross terms (lo@hi, hi@lo).
                terms = [(0, 0)] if nsplit == 1 else [(0, 0), (1, 0), (0, 1)]
                for k in range(KO):
                    for ti, (xi, wi) in enumerate(terms):
                        lhsT = xt_sb[:, xi, k, m * P:(m + 1) * P]
                        first = (k == 0 and ti == 0)
                        last = (k == KO - 1 and ti == len(terms) - 1)
                        for n in range(N_TILES):
                            rhs = w_sb[:, wi, k, n * N_TILE:(n + 1) * N_TILE]
                            nc.tensor.matmul(
                                ps[n][:],
                                lhsT=lhsT,
                                rhs=rhs,
                                start=first,
                                stop=last,
                            )
                for n in range(N_TILES):
                    nc.vector.tensor_add(
                        out_sb[:, n * N_TILE:(n + 1) * N_TILE],
                        ps[n][:],
                        bias_sb[:, n * N_TILE:(n + 1) * N_TILE],
                    )
                nc.sync.dma_start(out=out[m * P:(m + 1) * P, :], in_=out_sb[:])

    nc.compile()
    return nc


def _get_program(mode=None):
    mode = mode or MATMUL_MODE
    if mode not in _CACHE:
        _CACHE[mode] = _build_program(mode)
    return _CACHE[mode]


def _trunc12(a):
    """Truncate fp32 mantissa to 12 significand bits (fp32r-exact)."""
    return (a.view(np.uint32) & np.uint32(0xFFFFF000)).view(np.float32)


def _hi_lo(a):
    hi = _trunc12(a)
    return np.stack([hi, a - hi], axis=0)


def _shard_inputs(x, weights, bias, mode=None):
    mode = mode or MATMUL_MODE
    # Fold the constant block-diagonal mask into the weights on the host.
    col_block = np.arange(IO, dtype=np.int64) // OUT_SIZE
    mask = (col_block[None, :] != np.arange(IN_SIZE)[:, None])
    wm = weights * mask.astype(weights.dtype)
    xt = np.ascontiguousarray(x.T)
    if mode == "fp32x3":
        xt_in = _hi_lo(xt)
    else:
        xt_in = xt[None]
    in_maps = []
    for c in range(N_CORES):
        sl = slice(c * N_SHARD, (c + 1) * N_SHARD)
        w_shard = np.ascontiguousarray(wm[:, sl])
        if mode == "fp32x3":
            w_in = _hi_lo(w_shard)
        else:
            w_in = w_shard[None]
        in_maps.append({
            "xt": xt_in,
            "w": np.ascontiguousarray(w_in),
            "bias": np.ascontiguousarray(bias[sl]),
        })
    return in_maps


def run_sharded(in_maps, mode=None, **kwargs):
    """Run the SPMD program on cores 0-7. kwargs forwarded (e.g. trace)."""
    from concourse.bass_utils import run_bass_kernel_spmd

    nc = _get_program(mode)
    return run_bass_kernel_spmd(
        nc, in_maps, core_ids=list(range(N_CORES)), **kwargs
    )


def kernel(x: np.ndarray, weights: np.ndarray, bias: np.ndarray) -> np.ndarray:
    x = np.asarray(x, dtype=np.float32)
    weights = np.asarray(weights, dtype=np.float32)
    bias = np.asarray(bias, dtype=np.float32)
    in_maps = _shard_inputs(x, weights, bias)
    res = run_sharded(in_maps)
    full = np.concatenate([res.results[c]["out"] for c in range(N_CORES)], axis=1)
    return full.reshape(B, IN_SIZE, OUT_SIZE)


# revision 21
# speedup vs baseline: 1.8085x; 1.8085x over previous
"""Trainium2 Bass kernel for nn_ContinuousEmbedding (masked matmul + bias).

Computes out = x @ (weights * mask) + bias, reshaped to [B, in_size, out_size],
where mask zeroes each input feature's own [out_size]-wide diagonal block.

Strategy: tensor-parallel across the 8 NeuronCores by splitting the
in_size*out_size (=16384) output columns into 8 shards of 2048 columns.
Each core gets the full x (as x^T for the matmul's stationary operand),
its weight-column shard (mask is constant — folded into the weights on the
host), and its bias shard. Per core: out_shard = x @ W_shard + bias_shard
via 128x128 fp32 matmuls accumulating K=256 as 2 PSUM passes; bias-add is
fused into the PSUM->SBUF eviction on the vector engine.
"""

import numpy as np

B = 4096
IN_SIZE = 256
OUT_SIZE = 64
IO = IN_SIZE * OUT_SIZE          # 16384
N_CORES = 8
N_SHARD = IO // N_CORES          # 2048 output columns per core
P = 128                          # SBUF partitions
KO = IN_SIZE // P                # 2 contraction sub-tiles
N_TILE = 512                     # matmul moving free dim (fp32 max)
M_TILES = B // P                 # 32 output row tiles
N_TILES = N_SHARD // N_TILE      # 4 column tiles per core

MATMUL_MODE = "fp32r"            # "fp32" | "fp32r" | "fp32x3"

_CACHE: dict = {}


def _build_program(mode=None):
    mode = mode or MATMUL_MODE
    import concourse.mybir as mybir
    import concourse.tile as tile
    import concourse.bass as bass
    from concourse import bacc

    nsplit = 2 if mode == "fp32x3" else 1
    nc = bacc.Bacc(
        "TRN2", target_bir_lowering=False, debug=False, num_devices=N_CORES
    )
    xt = nc.dram_tensor(
        "xt", [nsplit, IN_SIZE, B], mybir.dt.float32, kind="ExternalInput"
    ).ap()
    w = nc.dram_tensor(
        "w", [nsplit, IN_SIZE, N_SHARD], mybir.dt.float32, kind="ExternalInput"
    ).ap()
    bias = nc.dram_tensor(
        "bias", [N_SHARD], mybir.dt.float32, kind="ExternalInput"
    ).ap()
    out = nc.dram_tensor(
        "out", [B, N_SHARD], mybir.dt.float32, kind="ExternalOutput"
    ).ap()

    with tile.TileContext(nc) as tc:
        with tc.tile_pool(name="const", bufs=1) as const, \
             tc.tile_pool(name="psum", bufs=2, space="PSUM") as psum_pool, \
             tc.tile_pool(name="outp", bufs=4) as outp:
            mm_dt = (mybir.dt.float32 if mode == "fp32"
                     else mybir.dt.float32r)
            w_sb = const.tile([P, nsplit, KO, N_SHARD], mm_dt)
            xt_sb = const.tile([P, nsplit, KO, B], mm_dt)
            bias_sb = const.tile([P, N_SHARD], mybir.dt.float32)
            bias_row = const.tile([1, N_SHARD], mybir.dt.float32)

            ld_eng = nc.sync if mode == "fp32" else nc.gpsimd
            # Weight load chunked by n-tile so the first matmuls start after
            # ~512KB instead of the full 2MB.
            w_src = w.rearrange("s (ko p) n -> p s ko n", p=P)
            for n in range(N_TILES):
                sl = slice(n * N_TILE, (n + 1) * N_TILE)
                ld_eng.dma_start(out=w_sb[:, :, :, sl], in_=w_src[:, :, :, sl])
            # x^T load: small first chunk so m-tile 0 can start immediately.
            xt_src = xt.rearrange("s (ko p) m -> p s ko m", p=P)
            bounds = [0, 256, 1024, 2048, 3072, B]
            for lo, hi in zip(bounds[:-1], bounds[1:]):
                sl = slice(lo, hi)
                ld_eng.dma_start(out=xt_sb[:, :, :, sl], in_=xt_src[:, :, :, sl])
            # bias: one 8KB HBM read, then SBUF->SBUF partition broadcast.
            nc.sync.dma_start(out=bias_row[:], in_=bias[None, :])
            nc.gpsimd.partition_broadcast(bias_sb[:], bias_row[:])

            # (x_split, w_split) matmul terms: plain modes use (0,0);
            # fp32x3 adds the hi/lo cross terms (lo@hi, hi@lo).
            terms = [(0, 0)] if nsplit == 1 else [(0, 0), (1, 0), (0, 1)]
            for m in range(M_TILES):
                out_sb = outp.tile([P, N_SHARD], mybir.dt.float32)
                for n in range(N_TILES):
                    ns = slice(n * N_TILE, (n + 1) * N_TILE)
                    ps = psum_pool.tile([P, N_TILE], mybir.dt.float32,
                                        name=f"ps{n}", tag=f"ps{n}")
                    nmm = KO * len(terms)
                    for i, (k, (xi, wi)) in enumerate(
                        (k, t) for k in range(KO) for t in terms
                    ):
                        nc.tensor.matmul(
                            ps[:],
                            lhsT=xt_sb[:, xi, k, m * P:(m + 1) * P],
                            rhs=w_sb[:, wi, k, ns],
                            start=(i == 0),
                            stop=(i == nmm - 1),
                        )
                    nc.vector.tensor_add(out_sb[:, ns], ps[:], bias_sb[:, ns])
                nc.sync.dma_start(out=out[m * P:(m + 1) * P, :], in_=out_sb[:])

    nc.compile()
    return nc


def _get_program(mode=None):
    mode = mode or MATMUL_MODE
    if mode not in _CACHE:
        _CACHE[mode] = _build_program(mode)
    return _CACHE[mode]


def _trunc12(a):
    """Truncate fp32 mantissa to 12 significand bits (fp32r-exact)."""
    return (a.view(np.uint32) & np.uint32(0xFFFFF000)).view(np.float32)


def _hi_lo(a):
    hi = _trunc12(a)
    return np.stack([hi, a - hi], axis=0)


def _shard_inputs(x, weights, bias, mode=None):
    mode = mode or MATMUL_MODE
    # Fold the constant block-diagonal mask into the weights on the host.
    col_block = np.arange(IO, dtype=np.int64) // OUT_SIZE
    mask = (col_block[None, :] != np.arange(IN_SIZE)[:, None])
    wm = weights * mask.astype(weights.dtype)
    xt = np.ascontiguousarray(x.T)
    if mode == "fp32x3":
        xt_in = _hi_lo(xt)
    else:
        xt_in = xt[None]
    in_maps = []
    for c in range(N_CORES):
        sl = slice(c * N_SHARD, (c + 1) * N_SHARD)
        w_shard = np.ascontiguousarray(wm[:, sl])
        if mode == "fp32x3":
            w_in = _hi_lo(w_shard)
        else:
            w_in = w_shard[None]
        in_maps.append({
            "xt": xt_in,
            "w": np.ascontiguousarray(w_in),
            "bias": np.ascontiguousarray(bias[sl]),
        })
    return in_maps


def run_sharded(in_maps, mode=None, **kwargs):
    """Run the SPMD program on cores 0-7. kwargs forwarded (e.g. trace)."""
    from concourse.bass_utils import run_bass_kernel_spmd

    nc = _get_program(mode)
    return run_bass_kernel_spmd(
        nc, in_maps, core_ids=list(range(N_CORES)), **kwargs
    )


def kernel(x: np.ndarray, weights: np.ndarray, bias: np.ndarray) -> np.ndarray:
    x = np.asarray(x, dtype=np.float32)
    weights = np.asarray(weights, dtype=np.float32)
    bias = np.asarray(bias, dtype=np.float32)
    in_maps = _shard_inputs(x, weights, bias)
    res = run_sharded(in_maps)
    full = np.concatenate([res.results[c]["out"] for c in range(N_CORES)], axis=1)
    return full.reshape(B, IN_SIZE, OUT_SIZE)
